# revision 1
# baseline (speedup 1.0000x reference)
"""EpisodicMemory retrieval kernel for Trainium2 (8 NeuronCores, data-parallel).

Reference computation (per row b of query):
    q = query @ Wq.T;  sim = l2norm(q) @ l2norm(keys).T
    top4 vals/idx;  w = softmax(5*vals);  retrieved = sum w_k * V[idx_k]
    projected = retrieved @ Wv.T
    gate = sigmoid([query, projected] @ Wg.T + bg);  out = gate * projected

Device mapping (per core, 2048 rows = 16 tiles of 128):
  - Selection path must match fp32 ranking: sim is computed as
    query @ Wc where Wc = Wq.T @ keys_norm.T is folded on host in fp64,
    and the matmul runs as a 3-pass fp32r hi/lo split (measured on HW:
    rms err 1.4e-7 == fp32-grade, at 1 cycle/col vs fp32's 4).
  - top-8 + indices via the DVE Max8/MaxIndex instructions.
  - V rows are gathered from HBM by index via the GPSIMD dma_gather
    (V pre-cast to fp16 on host; value-path tolerance is ~1e-2).
  - Weighted sum runs on the PE as 4 accumulated matmuls against
    diag(w_k), producing retrieved^T directly (feature-major) so the
    Wv / Wg matmuls need no transposes.
  - q itself is only needed for ||q|| (softmax temperature): one fp32r
    matmul + ACT square with accum_out.
  - All value-path matmuls use fp32r (~1e-4) with host/device-side
    fp32r rounding of operands.
"""

import sys

sys.path.insert(0, "/opt/trn_rl_repo")

import numpy as np
import ml_dtypes

from concourse import bass, bacc, mybir
from concourse.tile import TileContext
from concourse.bass_utils import run_bass_kernel_spmd

P = 128
D = 512
E = 1024
TOPK = 4
N_CORES = 8
B_FULL = 16384

DT = mybir.dt
F32 = DT.float32
F32R = DT.float32r
BF16 = DT.bfloat16
FP16 = DT.float16
I16 = DT.int16
U16 = DT.uint16

AF = mybir.ActivationFunctionType
ALU = mybir.AluOpType


def round_fp32r(x: np.ndarray) -> np.ndarray:
    """Round fp32 to the fp32r grid (1 sign + 8 exp + 11 mantissa bits, RNE).

    Matches TRN2's fp32_to_fp32r (verified bit-exact against the DVE/ACT
    hardware rounding on device).
    """
    u = np.ascontiguousarray(x, dtype=np.float32).view(np.uint32).astype(np.uint64)
    r = u + (0x7FF + ((u >> 12) & 1))
    r &= ~np.uint64(0xFFF)
    r = np.minimum(r, 0xFFFFFFFF).astype(np.uint32)
    return r.view(np.float32)


def build_program(nt: int, debug: bool = False):
    """Build the per-core bass program processing nt row-tiles of 128."""
    bc = nt * P  # rows per core
    tpg = 4 if nt % 4 == 0 else nt  # tiles per group (projT moving width)
    ng = nt // tpg
    gb = tpg * P  # rows per group

    nc = bacc.Bacc()

    qh_d = nc.declare_dram_parameter("qT_hi", [D, bc], F32R, isOutput=False)
    ql_d = nc.declare_dram_parameter("qT_lo", [D, bc], F32R, isOutput=False)
    wch_d = nc.declare_dram_parameter("Wc_hi", [D, E], F32R, isOutput=False)
    wcl_d = nc.declare_dram_parameter("Wc_lo", [D, E], F32R, isOutput=False)
    wqt_d = nc.declare_dram_parameter("WqT", [D, D], F32R, isOutput=False)
    wgt_d = nc.declare_dram_parameter("Wg1T", [D, D], F32R, isOutput=False)
    rep_d = nc.declare_dram_parameter("rep16", [16, P], F32R, isOutput=False)
    ident_d = nc.declare_dram_parameter("ident", [P, P], FP16, isOutput=False)
    vp_d = nc.declare_dram_parameter("Vp_hf", [E, D], FP16, isOutput=False)
    vg_d = nc.declare_dram_parameter("Vg_hf", [E, D], FP16, isOutput=False)

    out_d = nc.declare_dram_parameter("out", [bc, D], F32, isOutput=True)
    if debug:
        tpg_ = 4 if nt % 4 == 0 else nt
        dbg = {
            "d_top8": nc.declare_dram_parameter("d_top8", [nt, P, 8], F32, isOutput=True),
            "d_idx8": nc.declare_dram_parameter("d_idx8", [nt, P, 8], U16, isOutput=True),
            "d_gidx": nc.declare_dram_parameter("d_gidx", [nt, P, 32], I16, isOutput=True),
            "d_G": nc.declare_dram_parameter("d_G", [nt, P, TOPK, D], FP16, isOutput=True),
            "d_w": nc.declare_dram_parameter("d_w", [nt // tpg_, P, tpg_ * 4], F32, isOutput=True),
            "d_proj": nc.declare_dram_parameter("d_proj", [nt, P, D], F32, isOutput=True),
            "d_gate": nc.declare_dram_parameter("d_gate", [nt, P, D], F32, isOutput=True),
        }

    KC = D // P  # 4 contraction chunks of 128

    with TileContext(nc) as tc:
        with (
            tc.tile_pool(name="const", bufs=1) as cpool,
            tc.tile_pool(name="grp", bufs=2) as gpool,
            tc.tile_pool(name="work", bufs=2) as wpool,
            tc.tile_pool(name="dram", bufs=max(nt, 2), space="DRAM") as dpool,
            tc.tile_pool(name="ps_q", bufs=1, space="PSUM") as pp_q,
            tc.tile_pool(name="ps_s", bufs=3, space="PSUM") as pp_s,
            tc.tile_pool(name="ps_o", bufs=1, space="PSUM") as pp_o,
            tc.tile_pool(name="ps_z", bufs=1, space="PSUM") as pp_z,
        ):
            # ---- constants into SBUF ----
            # One strided DMA per tensor (eh-half for Wc), spread across
            # dispatch engines so the ~700ns/DMA sequencer cost parallelizes
            # and tile 0's matmuls start early.
            def load_const(eng, dram, rows, cols, dtype, name):
                sb = cpool.tile([P, rows // P, cols], dtype, tag=name)
                eng.dma_start(
                    out=sb, in_=dram.ap().rearrange("(c p) m -> p c m", p=P)
                )
                return sb

            wqt_sb = load_const(nc.sync, wqt_d, D, D, F32R, "wqt")
            wch_sb = cpool.tile([P, KC, E], F32R, tag="wch")
            wcl_sb = cpool.tile([P, KC, E], F32R, tag="wcl")
            for eh in range(2):
                es = slice(eh * D, (eh + 1) * D)
                nc.scalar.dma_start(
                    out=wch_sb[:, :, es],
                    in_=wch_d.ap()[:, es].rearrange("(c p) m -> p c m", p=P),
                )
                nc.scalar.dma_start(
                    out=wcl_sb[:, :, es],
                    in_=wcl_d.ap()[:, es].rearrange("(c p) m -> p c m", p=P),
                )
            wgt_sb = load_const(nc.sync, wgt_d, D, D, F32R, "wgt")
            ident_sb = cpool.tile([P, P], FP16, tag="ident")
            nc.sync.dma_start(out=ident_sb, in_=ident_d.ap())
            rep_sb = cpool.tile([16, P], F32R, tag="rep16")
            nc.sync.dma_start(out=rep_sb, in_=rep_d.ap())

            for g in range(ng):
                # ---- group input: queryT hi/lo chunks [128, KC, gb] ----
                qh_g = gpool.tile([P, KC, gb], F32R, tag="qh")
                ql_g = gpool.tile([P, KC, gb], F32R, tag="ql")
                nc.sync.dma_start(
                    out=qh_g,
                    in_=qh_d.ap()[:, g * gb : (g + 1) * gb].rearrange(
                        "(c p) m -> p c m", p=P
                    ),
                )
                nc.sync.dma_start(
                    out=ql_g,
                    in_=ql_d.ap()[:, g * gb : (g + 1) * gb].rearrange(
                        "(c p) m -> p c m", p=P
                    ),
                )

                normsq_g = gpool.tile([P, tpg], F32, tag="normsq")
                top8_g = gpool.tile([P, tpg * 8], F32, tag="top8")
                idx8_g = gpool.tile([P, tpg * 8], U16, tag="idx8")
                g_tiles = []

                for t in range(tpg):
                    ti = g * tpg + t
                    bs = slice(t * P, (t + 1) * P)

                    # ---- q = query @ Wq.T (fp32r, for the norm only) ----
                    ps_q = pp_q.tile([P, D], F32, tag="q")
                    for c in range(KC):
                        nc.tensor.matmul(
                            ps_q,
                            qh_g[:, c, bs],
                            wqt_sb[:, c, :],
                            start=(c == 0),
                            stop=(c == KC - 1),
                        )
                    qsq = wpool.tile([P, D], F32, tag="qsq")
                    nc.scalar.activation(
                        qsq, ps_q, AF.Square, accum_out=normsq_g[:, t : t + 1]
                    )

                    # ---- sim = query @ Wc  (fp32r hi/lo 3-pass) ----
                    sim_sb = wpool.tile([P, E], F32, tag="sim")
                    for eh in range(2):
                        ps_s = pp_s.tile([P, D], F32, tag="s")
                        es = slice(eh * D, (eh + 1) * D)
                        i = 0
                        for c in range(KC):
                            for qa, wb in (
                                (qh_g, wch_sb),
                                (qh_g, wcl_sb),
                                (ql_g, wch_sb),
                            ):
                                nc.tensor.matmul(
                                    ps_s,
                                    qa[:, c, bs],
                                    wb[:, c, es],
                                    start=(i == 0),
                                    stop=(i == 3 * KC - 1),
                                )
                                i += 1
                        if eh == 0:
                            nc.scalar.copy(sim_sb[:, es], ps_s)
                        else:
                            nc.vector.tensor_copy(sim_sb[:, es], ps_s)

                    # ---- top-8 values + indices ----
                    t8 = slice(t * 8, (t + 1) * 8)
                    nc.vector.max(out=top8_g[:, t8], in_=sim_sb)
                    nc.vector.max_index(
                        out=idx8_g[:, t8], in_max=top8_g[:, t8], in_values=sim_sb
                    )

                    # ---- index shuffle [128b, 4k] -> [16, 32] via DRAM bounce ----
                    # Shuffle [128b, 4k] -> gather layout: element i=k*128+b
                    # reads its index at [b%16, k*8 + b//16]; the 16-partition
                    # block is replicated to all 8 GPSIMD core groups.
                    # The idx shuffle needs a DRAM bounce (the partition
                    # permutation b -> b%16 is not expressible in one SBUF
                    # DMA). Indices travel as exact fp32r integers so the 8x
                    # replication for the GPSIMD cores can run on the PE
                    # (all orderings are engine-sem or SWDGE edges, which
                    # are reliable; HWDGE fan-out accounting is not).
                    idxf = wpool.tile([P, 4], F32R, tag="idxf")
                    nc.vector.tensor_copy(idxf, idx8_g[:, t * 8 : t * 8 + 4])
                    scratch = dpool.tile([1, 512], F32R, tag="scr")
                    # hop1 goes through the GPSIMD SWDGE: its completion-sem
                    # accounting is per-instruction-exact (HWDGE fan-out
                    # accounting is not, for scattered writes like this).
                    nc.gpsimd.dma_start(
                        out=scratch.rearrange("o (c k h) -> o h c k", c=16, k=4, h=8),
                        in_=idxf,
                    )
                    gidxf16 = wpool.tile([16, 32], F32R, tag="gidxf16")
                    nc.sync.dma_start(out=gidxf16, in_=scratch)
                    ps_g = pp_q.tile([P, 32], F32, tag="gidx")
                    nc.tensor.matmul(ps_g, rep_sb, gidxf16, start=True, stop=True)
                    gidx = wpool.tile([P, 32], I16, tag="gidx")
                    nc.vector.tensor_copy(gidx, ps_g)

                    # ---- gather pre-projected value rows (fp16) ----
                    # Vp = V @ Wv.T and Vg = Vp @ Wg2.T are folded on host,
                    # so the weighted sums below produce `projected` and the
                    # gate's Z2 term directly -- no Wv/projT stages.
                    gp_t = wpool.tile([P, TOPK, D], FP16, tag="Gp")
                    nc.gpsimd.dma_gather(
                        out_ap=gp_t,
                        in_ap=vp_d.ap(),
                        idxs_ap=gidx,
                        num_idxs=TOPK * P,
                        num_idxs_reg=TOPK * P,
                        elem_size=D,
                    )
                    gg_t = wpool.tile([P, TOPK, D], FP16, tag="Gg")
                    nc.gpsimd.dma_gather(
                        out_ap=gg_t,
                        in_ap=vg_d.ap(),
                        idxs_ap=gidx,
                        num_idxs=TOPK * P,
                        num_idxs_reg=TOPK * P,
                        elem_size=D,
                    )
                    g_tiles.append((gp_t, gg_t))
                    if debug:
                        nc.sync.dma_start(out=dbg["d_top8"].ap()[ti], in_=top8_g[:, t8])
                        nc.sync.dma_start(out=dbg["d_idx8"].ap()[ti], in_=idx8_g[:, t8])
                        nc.sync.dma_start(out=dbg["d_gidx"].ap()[ti], in_=gidx)
                        nc.sync.dma_start(out=dbg["d_G"].ap()[ti], in_=gp_t)

                # ---- softmax over top-4 (batched across the group) ----
                nrm = gpool.tile([P, tpg], F32, tag="nrm")
                nc.scalar.sqrt(nrm, normsq_g)
                rrec = gpool.tile([P, tpg], F32, tag="rrec")
                nc.vector.reciprocal(rrec, nrm)
                s5 = gpool.tile([P, tpg], F32, tag="s5")
                nc.vector.tensor_scalar_mul(s5, rrec, 5.0)

                t8v = top8_g.rearrange("p (t k) -> p t k", k=8)
                top4_v = t8v[:, :, 0:4]
                m_v = t8v[:, :, 0:1].to_broadcast([P, tpg, 4])
                s5_v = s5.rearrange("p (t o) -> p t o", o=1).to_broadcast([P, tpg, 4])

                args = gpool.tile([P, tpg * 4], F32, tag="args")
                args_v = args.rearrange("p (t k) -> p t k", k=4)
                nc.vector.tensor_tensor(args_v, top4_v, m_v, op=ALU.subtract)
                nc.vector.tensor_tensor(args_v, args_v, s5_v, op=ALU.mult)
                ex = gpool.tile([P, tpg * 4], F32, tag="ex")
                nc.scalar.activation(ex, args, AF.Exp)
                ex_v = ex.rearrange("p (t k) -> p t k", k=4)
                den = gpool.tile([P, tpg], F32, tag="den")
                nc.vector.tensor_reduce(den, ex_v, axis=mybir.AxisListType.X, op=ALU.add)
                rden = gpool.tile([P, tpg], F32, tag="rden")
                nc.vector.reciprocal(rden, den)
                rden_v = rden.rearrange("p (t o) -> p t o", o=1).to_broadcast(
                    [P, tpg, 4]
                )
                w_g = gpool.tile([P, tpg * 4], F32, tag="w")
                w_v = w_g.rearrange("p (t k) -> p t k", k=4)
                nc.vector.tensor_tensor(w_v, ex_v, rden_v, op=ALU.mult)
                if debug:
                    nc.sync.dma_start(out=dbg["d_w"].ap()[g], in_=w_g)

                # ---- per tile: diag(w_k) matmuls -> projected + gate ----
                for t in range(tpg):
                    ti = g * tpg + t
                    bs = slice(t * P, (t + 1) * P)
                    gp_t, gg_t = g_tiles[t]

                    diag4 = wpool.tile([P, TOPK, P], FP16, tag="diag4")
                    for k in range(TOPK):
                        nc.vector.tensor_scalar_mul(
                            diag4[:, k, :], ident_sb, w_g[:, t * 4 + k : t * 4 + k + 1]
                        )

                    # projected[b, :] = sum_k w_k[b] * Vp[idx_k[b], :]
                    ps_o = pp_o.tile([P, D], F32, tag="o")
                    for k in range(TOPK):
                        nc.tensor.matmul(
                            ps_o,
                            diag4[:, k, :],
                            gp_t[:, k, :],
                            start=(k == 0),
                            stop=(k == TOPK - 1),
                        )
                    proj_sb = wpool.tile([P, D], F32, tag="proj")
                    nc.scalar.copy(proj_sb, ps_o)
                    if debug:
                        nc.sync.dma_start(out=dbg["d_proj"].ap()[ti], in_=proj_sb)

                    # Z = query @ Wg1.T + sum_k w_k * (Vg[idx_k] + bg)
                    # (bg is folded into Vg on host; softmax weights sum to 1)
                    ps_z = pp_z.tile([P, D], F32, tag="z")
                    for kc in range(KC):
                        nc.tensor.matmul(
                            ps_z,
                            qh_g[:, kc, bs],
                            wgt_sb[:, kc, :],
                            start=(kc == 0),
                            stop=False,
                        )
                    for k in range(TOPK):
                        nc.tensor.matmul(
                            ps_z,
                            diag4[:, k, :],
                            gg_t[:, k, :],
                            start=False,
                            stop=(k == TOPK - 1),
                        )
                    gate_sb = wpool.tile([P, D], F32, tag="gate")
                    nc.scalar.activation(gate_sb, ps_z, AF.Sigmoid)
                    if debug:
                        nc.sync.dma_start(out=dbg["d_gate"].ap()[ti], in_=gate_sb)

                    out_sb = wpool.tile([P, D], F32, tag="outb")
                    nc.vector.tensor_mul(out_sb, gate_sb, proj_sb)
                    nc.sync.dma_start(
                        out=out_d.ap()[ti * P : (ti + 1) * P, :], in_=out_sb
                    )

    nc.compile()
    return nc


def _host_prep(query, episode_keys, episode_values, Wq, Wv, Wg, bg):
    """Fold constants in fp64 and stage per-core device inputs."""
    kn = episode_keys.astype(np.float64)
    kn = kn / np.maximum(np.linalg.norm(kn, axis=1, keepdims=True), 1e-12)
    wc64 = Wq.astype(np.float64).T @ kn.T  # [D, E]
    wc_hi = round_fp32r(wc64.astype(np.float32))
    wc_lo = round_fp32r((wc64 - wc_hi.astype(np.float64)).astype(np.float32))

    q = np.ascontiguousarray(query, dtype=np.float32)
    q_hi = round_fp32r(q)
    q_lo = round_fp32r(q - q_hi)
    qT_hi = np.ascontiguousarray(q_hi.T)  # [D, B]
    qT_lo = np.ascontiguousarray(q_lo.T)

    v64 = episode_values.astype(np.float64)
    vp64 = v64 @ Wv.astype(np.float64).T                  # projected values
    vg64 = vp64 @ Wg.astype(np.float64)[:, D:].T          # gate Z2 values
    vg64 = vg64 + bg.astype(np.float64)[None, :]          # bg folded (sum w = 1)
    consts = {
        "Wc_hi": np.ascontiguousarray(wc_hi),
        "Wc_lo": np.ascontiguousarray(wc_lo),
        "WqT": round_fp32r(np.ascontiguousarray(Wq.T)),
        "Wg1T": round_fp32r(np.ascontiguousarray(Wg.T[:D])),
        "ident": np.eye(P, dtype=np.float16),
        "rep16": np.tile(np.eye(16, dtype=np.float32), (1, P // 16)).reshape(16, P),
        "Vp_hf": vp64.astype(np.float16),
        "Vg_hf": vg64.astype(np.float16),
    }
    return qT_hi, qT_lo, consts


_PROGRAM_CACHE: dict = {}


def kernel(query, episode_keys, episode_values, Wq, Wv, Wg, bg, top_k):
    assert int(top_k) == TOPK
    query = np.asarray(query, dtype=np.float32)
    assert query.shape == (B_FULL, D), query.shape

    nt = B_FULL // N_CORES // P  # 16 tiles per core
    if nt not in _PROGRAM_CACHE:
        _PROGRAM_CACHE[nt] = build_program(nt)
    nc = _PROGRAM_CACHE[nt]

    qT_hi, qT_lo, consts = _host_prep(
        query,
        np.asarray(episode_keys, dtype=np.float32),
        np.asarray(episode_values, dtype=np.float32),
        np.asarray(Wq, dtype=np.float32),
        np.asarray(Wv, dtype=np.float32),
        np.asarray(Wg, dtype=np.float32),
        np.asarray(bg, dtype=np.float32),
    )

    bc = B_FULL // N_CORES
    in_maps = []
    for c in range(N_CORES):
        m = dict(consts)
        m["qT_hi"] = np.ascontiguousarray(qT_hi[:, c * bc : (c + 1) * bc])
        m["qT_lo"] = np.ascontiguousarray(qT_lo[:, c * bc : (c + 1) * bc])
        in_maps.append(m)

    res = run_bass_kernel_spmd(nc, in_maps, list(range(N_CORES)))
    global _LAST_RUN
    _LAST_RUN = res
    out = np.concatenate([res.results[c]["out"] for c in range(N_CORES)], axis=0)
    return out.astype(np.float32)


_LAST_RUN = None



# revision 4
# speedup vs baseline: 1.5273x; 1.5273x over previous
"""EpisodicMemory retrieval kernel for Trainium2 (8 NeuronCores, data-parallel).

Reference computation (per row b of query):
    q = query @ Wq.T;  sim = l2norm(q) @ l2norm(keys).T
    top4 vals/idx;  w = softmax(5*vals);  retrieved = sum w_k * V[idx_k]
    projected = retrieved @ Wv.T
    gate = sigmoid([query, projected] @ Wg.T + bg);  out = gate * projected

Device mapping (per core, 2048 rows = 16 tiles of 128):
  - Selection path must preserve the fp32 ranking: sim = query @ Wc with
    Wc = Wq.T @ keys_norm.T folded on host in fp64. The matmul runs as a
    hybrid: one fp32r hi*hi pass (1 cyc/col) plus two fp8 DoubleRow
    residual passes (0.5 cyc/col, 256-deep) capturing lo*hi and hi*lo:
        sim ~= qh.wh + e4m3(ql*2^8).e5m2(wh*2^-8) + e5m2(qh*2^-8).e5m2(wl*2^8)
    The fp8 operand pre-scaling keeps every product at natural scale so
    all passes accumulate into one PSUM bank. Residual noise ~5e-7 on
    cosine sims, a handful of 4-vs-5 boundary swaps over the full batch.
  - top-8 + indices via the DVE Max8/MaxIndex instructions.
  - Vp = V @ Wv.T and Vg = Vp @ Wg2.T + bg are folded on host into one
    fp16 table [E, 2D], gathered per row by the GPSIMD dma_gather as
    int32 words (halves the per-element gather cost on the Pool queue).
  - Weighted sums run on the PE as 4 accumulated fp16 matmuls against
    diag(w_k), producing `projected` and the gate's Z2 term directly.
  - q itself is only needed for ||q|| (softmax temperature): one fp8
    DoubleRow matmul + ACT square with accum_out.
  - Output path (gate, projected, out) is fp16: DVE runs the final
    multiply in 2x mode and the store DMA halves; host casts to fp32.
"""

import sys

sys.path.insert(0, "/opt/trn_rl_repo")

import numpy as np
import ml_dtypes

from concourse import bass, bacc, mybir
from concourse.tile import TileContext
from concourse.bass_utils import run_bass_kernel_spmd

P = 128
D = 512
E = 1024
TOPK = 4
N_CORES = 8
B_FULL = 16384

DT = mybir.dt
F32 = DT.float32
F32R = DT.float32r
FP16 = DT.float16
F8E4 = DT.float8e4
F8E5 = DT.float8e5
I16 = DT.int16
U16 = DT.uint16
I64 = DT.int64

E4 = ml_dtypes.float8_e4m3
E5 = ml_dtypes.float8_e5m2

AF = mybir.ActivationFunctionType
ALU = mybir.AluOpType
DR = mybir.MatmulPerfMode.DoubleRow

# fp8 residual-pass operand pre-scaling (exact powers of two, product = 1)
FS = 256.0


def round_fp32r(x: np.ndarray) -> np.ndarray:
    """Round fp32 to the fp32r grid (1 sign + 8 exp + 11 mantissa bits, RNE)."""
    u = np.ascontiguousarray(x, dtype=np.float32).view(np.uint32).astype(np.uint64)
    r = u + (0x7FF + ((u >> 12) & 1))
    r &= ~np.uint64(0xFFF)
    r = np.minimum(r, 0xFFFFFFFF).astype(np.uint32)
    return r.view(np.float32)


def build_program(nt: int):
    """Build the per-core bass program processing nt row-tiles of 128."""
    bc = nt * P  # rows per core
    tpg = 4 if nt % 4 == 0 else nt  # tiles per group
    ng = nt // tpg
    gb = tpg * P  # rows per group

    nc = bacc.Bacc()

    qh_d = nc.declare_dram_parameter("qT_hi", [D, bc], F32R, isOutput=False)
    qls_d = nc.declare_dram_parameter("qT_ls", [D, bc], F8E4, isOutput=False)
    qhs_d = nc.declare_dram_parameter("qT_hs", [D, bc], F8E5, isOutput=False)
    q8_d = nc.declare_dram_parameter("qT_8", [D, bc], F8E4, isOutput=False)
    wch_d = nc.declare_dram_parameter("Wc_hi", [D, E], F32R, isOutput=False)
    whs_d = nc.declare_dram_parameter("Wc_hs", [D, E], F8E5, isOutput=False)
    wls_d = nc.declare_dram_parameter("Wc_ls", [D, E], F8E5, isOutput=False)
    wq8_d = nc.declare_dram_parameter("WqT8", [D, D], F8E4, isOutput=False)
    wgt_d = nc.declare_dram_parameter("Wg1T", [D, D], F32R, isOutput=False)
    rep_d = nc.declare_dram_parameter("rep16", [16, P], F32R, isOutput=False)
    ident_d = nc.declare_dram_parameter("ident", [P, P], FP16, isOutput=False)
    vpg_d = nc.declare_dram_parameter("Vpg32", [E, D], DT.int32, isOutput=False)

    out_d = nc.declare_dram_parameter("out", [bc, D], FP16, isOutput=True)

    KC = D // P  # 4 contraction chunks of 128 for fp32r passes

    with TileContext(nc) as tc:
        with (
            tc.tile_pool(name="const", bufs=1) as cpool,
            tc.tile_pool(name="grp", bufs=2) as gpool,
            tc.tile_pool(name="work", bufs=2) as wpool,
            tc.tile_pool(name="dram", bufs=max(nt, 2), space="DRAM") as dpool,
            tc.tile_pool(name="ps_q", bufs=1, space="PSUM") as pp_q,
            tc.tile_pool(name="ps_s", bufs=3, space="PSUM") as pp_s,
            tc.tile_pool(name="ps_o", bufs=1, space="PSUM") as pp_o,
            tc.tile_pool(name="ps_z", bufs=1, space="PSUM") as pp_z,
        ):
            # ---- constants into SBUF ----
            # Spread across dispatch queues; load what tile 0 needs first
            # (wch half 0 on scalar, whs/wls half 0 on vector).
            wch_sb = cpool.tile([P, KC, E], F32R, tag="wch")
            whs_sb = cpool.tile([P, 2, 2, E], F8E5, tag="whs")
            wls_sb = cpool.tile([P, 2, 2, E], F8E5, tag="wls")
            wq8_sb = cpool.tile([P, 2, 2, D], F8E4, tag="wq8")
            wgt_sb = cpool.tile([P, KC, D], F32R, tag="wgt")
            for eh in range(2):
                es = slice(eh * D, (eh + 1) * D)
                nc.scalar.dma_start(
                    out=wch_sb[:, :, es],
                    in_=wch_d.ap()[:, es].rearrange("(c p) m -> p c m", p=P),
                )
                nc.gpsimd.dma_start(
                    out=whs_sb[:, :, :, es],
                    in_=whs_d.ap()[:, es].rearrange("(c t p) m -> p c t m", c=2, t=2),
                )
                nc.gpsimd.dma_start(
                    out=wls_sb[:, :, :, es],
                    in_=wls_d.ap()[:, es].rearrange("(c t p) m -> p c t m", c=2, t=2),
                )
                if eh == 0:
                    nc.gpsimd.dma_start(
                        out=wq8_sb,
                        in_=wq8_d.ap().rearrange("(c t p) m -> p c t m", c=2, t=2),
                    )
            nc.gpsimd.dma_start(
                out=wgt_sb, in_=wgt_d.ap().rearrange("(c p) m -> p c m", p=P)
            )
            ident_sb = cpool.tile([P, P], FP16, tag="ident")
            nc.sync.dma_start(out=ident_sb, in_=ident_d.ap())
            rep_sb = cpool.tile([16, P], F32R, tag="rep16")
            nc.sync.dma_start(out=rep_sb, in_=rep_d.ap())

            for g in range(ng):
                gs = slice(g * gb, (g + 1) * gb)
                # ---- group inputs ----
                qh_g = gpool.tile([P, KC, gb], F32R, tag="qh")
                nc.sync.dma_start(
                    out=qh_g, in_=qh_d.ap()[:, gs].rearrange("(c p) m -> p c m", p=P)
                )
                qls_g = gpool.tile([P, 2, 2, gb], F8E4, tag="qls")
                nc.sync.dma_start(
                    out=qls_g,
                    in_=qls_d.ap()[:, gs].rearrange("(c t p) m -> p c t m", c=2, t=2),
                )
                qhs_g = gpool.tile([P, 2, 2, gb], F8E5, tag="qhs")
                nc.sync.dma_start(
                    out=qhs_g,
                    in_=qhs_d.ap()[:, gs].rearrange("(c t p) m -> p c t m", c=2, t=2),
                )
                q8_g = gpool.tile([P, 2, 2, gb], F8E4, tag="q8")
                nc.sync.dma_start(
                    out=q8_g,
                    in_=q8_d.ap()[:, gs].rearrange("(c t p) m -> p c t m", c=2, t=2),
                )

                normsq_g = gpool.tile([P, tpg], F32, tag="normsq")
                top8_g = gpool.tile([P, tpg * 8], F32, tag="top8")
                idx8_g = gpool.tile([P, tpg * 8], U16, tag="idx8")
                g_tiles = []

                for t in range(tpg):
                    ti = g * tpg + t
                    bs = slice(t * P, (t + 1) * P)

                    # ---- ||q||^2 via fp8 DoubleRow q = query @ Wq.T ----
                    ps_q = pp_q.tile([P, D], F32, tag="q")
                    for cc in range(2):
                        nc.tensor.matmul(
                            ps_q,
                            q8_g[:, cc, :, bs],
                            wq8_sb[:, cc, :, :],
                            start=(cc == 0),
                            stop=(cc == 1),
                            perf_mode=DR,
                        )
                    qsq = wpool.tile([P, D], F32, tag="qsq")
                    nc.scalar.activation(
                        qsq, ps_q, AF.Square, accum_out=normsq_g[:, t : t + 1]
                    )

                    # ---- sim = query @ Wc (fp32r hi*hi + fp8 DR residuals) ----
                    sim_sb = wpool.tile([P, E], F32, tag="sim")
                    for eh in range(2):
                        ps_s = pp_s.tile([P, D], F32, tag="s")
                        es = slice(eh * D, (eh + 1) * D)
                        for c in range(KC):
                            nc.tensor.matmul(
                                ps_s,
                                qh_g[:, c, bs],
                                wch_sb[:, c, es],
                                start=(c == 0),
                                stop=False,
                            )
                        for cc in range(2):
                            nc.tensor.matmul(
                                ps_s,
                                qls_g[:, cc, :, bs],
                                whs_sb[:, cc, :, es],
                                start=False,
                                stop=False,
                                perf_mode=DR,
                            )
                        for cc in range(2):
                            nc.tensor.matmul(
                                ps_s,
                                qhs_g[:, cc, :, bs],
                                wls_sb[:, cc, :, es],
                                start=False,
                                stop=(cc == 1),
                                perf_mode=DR,
                            )
                        nc.scalar.copy(sim_sb[:, es], ps_s)

                    # ---- top-8 values + indices ----
                    t8 = slice(t * 8, (t + 1) * 8)
                    nc.vector.max(out=top8_g[:, t8], in_=sim_sb)
                    nc.vector.max_index(
                        out=idx8_g[:, t8], in_max=top8_g[:, t8], in_values=sim_sb
                    )

                    # ---- index shuffle [128b, 4k] -> [16, 32] via DRAM bounce ----
                    # (see baseline notes: the b -> b%16 partition permutation
                    # needs a DRAM hop; indices travel as exact fp32r ints so
                    # the 8x replication for the GPSIMD cores runs on the PE)
                    idxf = wpool.tile([P, 4], F32R, tag="idxf")
                    nc.vector.tensor_copy(idxf, idx8_g[:, t * 8 : t * 8 + 4])
                    scratch = dpool.tile([1, 512], F32R, tag="scr")
                    nc.gpsimd.dma_start(
                        out=scratch.rearrange("o (c k h) -> o h c k", c=16, k=4, h=8),
                        in_=idxf,
                    )
                    gidxf16 = wpool.tile([16, 32], F32R, tag="gidxf16")
                    nc.sync.dma_start(out=gidxf16, in_=scratch)
                    ps_g = pp_q.tile([P, 32], F32, tag="gidx")
                    nc.tensor.matmul(ps_g, rep_sb, gidxf16, start=True, stop=True)
                    gidx = wpool.tile([P, 32], I16, tag="gidx")
                    nc.vector.tensor_copy(gidx, ps_g)

                    # ---- gather merged [Vp | Vg] rows as int64 words ----
                    g64 = wpool.tile([P, TOPK, D], DT.int32, tag="G32")
                    nc.gpsimd.dma_gather(
                        out_ap=g64,
                        in_ap=vpg_d.ap(),
                        idxs_ap=gidx,
                        num_idxs=TOPK * P,
                        num_idxs_reg=TOPK * P,
                        elem_size=D,
                    )
                    g_tiles.append(g64)

                # ---- softmax over top-4 (batched across the group) ----
                nrm = gpool.tile([P, tpg], F32, tag="nrm")
                nc.scalar.sqrt(nrm, normsq_g)
                rrec = gpool.tile([P, tpg], F32, tag="rrec")
                nc.vector.reciprocal(rrec, nrm)
                s5 = gpool.tile([P, tpg], F32, tag="s5")
                nc.vector.tensor_scalar_mul(s5, rrec, 5.0)

                t8v = top8_g.rearrange("p (t k) -> p t k", k=8)
                top4_v = t8v[:, :, 0:4]
                m_v = t8v[:, :, 0:1].to_broadcast([P, tpg, 4])
                s5_v = s5.rearrange("p (t o) -> p t o", o=1).to_broadcast([P, tpg, 4])

                args = gpool.tile([P, tpg * 4], F32, tag="args")
                args_v = args.rearrange("p (t k) -> p t k", k=4)
                nc.vector.tensor_tensor(args_v, top4_v, m_v, op=ALU.subtract)
                nc.vector.tensor_tensor(args_v, args_v, s5_v, op=ALU.mult)
                ex = gpool.tile([P, tpg * 4], F32, tag="ex")
                nc.scalar.activation(ex, args, AF.Exp)
                ex_v = ex.rearrange("p (t k) -> p t k", k=4)
                den = gpool.tile([P, tpg], F32, tag="den")
                nc.vector.tensor_reduce(den, ex_v, axis=mybir.AxisListType.X, op=ALU.add)
                rden = gpool.tile([P, tpg], F32, tag="rden")
                nc.vector.reciprocal(rden, den)
                rden_v = rden.rearrange("p (t o) -> p t o", o=1).to_broadcast(
                    [P, tpg, 4]
                )
                w_g = gpool.tile([P, tpg * 4], F32, tag="w")
                w_v = w_g.rearrange("p (t k) -> p t k", k=4)
                nc.vector.tensor_tensor(w_v, ex_v, rden_v, op=ALU.mult)

                # ---- per tile: diag(w_k) matmuls -> projected + gate ----
                for t in range(tpg):
                    ti = g * tpg + t
                    bs = slice(t * P, (t + 1) * P)
                    g16 = g_tiles[t].bitcast(FP16)  # [P, TOPK, 2D]

                    diag4 = wpool.tile([P, TOPK, P], FP16, tag="diag4")
                    for k in range(TOPK):
                        nc.vector.tensor_scalar_mul(
                            diag4[:, k, :], ident_sb, w_g[:, t * 4 + k : t * 4 + k + 1]
                        )

                    # projected[b, :] = sum_k w_k[b] * Vp[idx_k[b], :]
                    ps_o = pp_o.tile([P, D], F32, tag="o")
                    for k in range(TOPK):
                        nc.tensor.matmul(
                            ps_o,
                            diag4[:, k, :],
                            g16[:, k, 0:D],
                            start=(k == 0),
                            stop=(k == TOPK - 1),
                        )
                    proj_sb = wpool.tile([P, D], FP16, tag="proj")
                    nc.scalar.copy(proj_sb, ps_o)

                    # Z = query @ Wg1.T + sum_k w_k * (Vg[idx_k] + bg)
                    ps_z = pp_z.tile([P, D], F32, tag="z")
                    for kc in range(KC):
                        nc.tensor.matmul(
                            ps_z,
                            qh_g[:, kc, bs],
                            wgt_sb[:, kc, :],
                            start=(kc == 0),
                            stop=False,
                        )
                    for k in range(TOPK):
                        nc.tensor.matmul(
                            ps_z,
                            diag4[:, k, :],
                            g16[:, k, D : 2 * D],
                            start=False,
                            stop=(k == TOPK - 1),
                        )
                    gate_sb = wpool.tile([P, D], FP16, tag="gate")
                    nc.scalar.activation(gate_sb, ps_z, AF.Sigmoid)

                    out_sb = wpool.tile([P, D], FP16, tag="outb")
                    nc.vector.tensor_mul(out_sb, gate_sb, proj_sb)
                    nc.sync.dma_start(
                        out=out_d.ap()[ti * P : (ti + 1) * P, :], in_=out_sb
                    )

    nc.compile()
    return nc


def _host_prep(query, episode_keys, episode_values, Wq, Wv, Wg, bg):
    """Fold constants in fp64 and stage per-core device inputs."""
    kn = episode_keys.astype(np.float64)
    kn = kn / np.maximum(np.linalg.norm(kn, axis=1, keepdims=True), 1e-12)
    wc64 = Wq.astype(np.float64).T @ kn.T  # [D, E]
    wc_hi = round_fp32r(wc64.astype(np.float32))
    wc_lo = (wc64 - wc_hi.astype(np.float64)).astype(np.float32)

    q = np.ascontiguousarray(query, dtype=np.float32)
    q_hi = round_fp32r(q)
    q_lo = q - q_hi
    qT_hi = np.ascontiguousarray(q_hi.T)  # [D, B]
    qT_ls = np.ascontiguousarray((q_lo.T * FS)).astype(E4)
    qT_hs = np.ascontiguousarray((qT_hi * (1.0 / FS))).astype(E5)
    qT_8 = np.ascontiguousarray(q.T).astype(E4)

    v64 = episode_values.astype(np.float64)
    vp64 = v64 @ Wv.astype(np.float64).T                  # projected values
    vg64 = vp64 @ Wg.astype(np.float64)[:, D:].T          # gate Z2 values
    vg64 = vg64 + bg.astype(np.float64)[None, :]          # bg folded (sum w = 1)
    vpg16 = np.ascontiguousarray(
        np.concatenate(
            [vp64.astype(np.float16), vg64.astype(np.float16)], axis=1
        )
    )  # [E, 2D] fp16
    consts = {
        "Wc_hi": np.ascontiguousarray(wc_hi),
        "Wc_hs": (wc_hi * (1.0 / FS)).astype(E5),
        "Wc_ls": (wc_lo * FS).astype(E5),
        "WqT8": np.ascontiguousarray(Wq.T).astype(E4),
        "Wg1T": round_fp32r(np.ascontiguousarray(Wg.T[:D])),
        "ident": np.eye(P, dtype=np.float16),
        "rep16": np.tile(np.eye(16, dtype=np.float32), (1, P // 16)).reshape(16, P),
        "Vpg32": vpg16.view(np.int32),
    }
    return qT_hi, qT_ls, qT_hs, qT_8, consts


_PROGRAM_CACHE: dict = {}


def kernel(query, episode_keys, episode_values, Wq, Wv, Wg, bg, top_k):
    assert int(top_k) == TOPK
    query = np.asarray(query, dtype=np.float32)
    assert query.shape == (B_FULL, D), query.shape

    nt = B_FULL // N_CORES // P  # 16 tiles per core
    if nt not in _PROGRAM_CACHE:
        _PROGRAM_CACHE[nt] = build_program(nt)
    nc = _PROGRAM_CACHE[nt]

    qT_hi, qT_ls, qT_hs, qT_8, consts = _host_prep(
        query,
        np.asarray(episode_keys, dtype=np.float32),
        np.asarray(episode_values, dtype=np.float32),
        np.asarray(Wq, dtype=np.float32),
        np.asarray(Wv, dtype=np.float32),
        np.asarray(Wg, dtype=np.float32),
        np.asarray(bg, dtype=np.float32),
    )

    bc = B_FULL // N_CORES
    in_maps = []
    for c in range(N_CORES):
        cs = slice(c * bc, (c + 1) * bc)
        m = dict(consts)
        m["qT_hi"] = np.ascontiguousarray(qT_hi[:, cs])
        m["qT_ls"] = np.ascontiguousarray(qT_ls[:, cs])
        m["qT_hs"] = np.ascontiguousarray(qT_hs[:, cs])
        m["qT_8"] = np.ascontiguousarray(qT_8[:, cs])
        in_maps.append(m)

    res = run_bass_kernel_spmd(nc, in_maps, list(range(N_CORES)))
    global _LAST_RUN
    _LAST_RUN = res
    out = np.concatenate([res.results[c]["out"] for c in range(N_CORES)], axis=0)
    return out.astype(np.float32)


_LAST_RUN = None
